# revision 28
# baseline (speedup 1.0000x reference)
"""DeepTEN encoding kernel for Trainium2 (8 NeuronCores, SPMD data-parallel over batch).

Math (per batch b):
    xf = x[b] viewed (D, N), N = H*W
    dist[n,k] = ||xf[:,n] - c[k]||^2 ;  logits = -scale * dist ;  A = softmax_k(logits)
    E[k,d] = sum_n A[n,k] * (xf[d,n] - c[k,d]) = (A^T X)[k,d] - colsum(A)[k]*c[k,d]

Device decomposition (softmax in (n-partitions, k-free) layout):
    w = -scale (>0), maxs = max(w)
    PSUM(xc)[n,k] = -2*w_k*<x_n,c_k> + w_k*csq_k      (x-tile-stationary matmuls +
                                                       a rank-1 seed matmul)
    G[n,k] = exp((w_k-maxs)*x_sq[n])                  (DVE mult + ACT exp; x_sq is
                                                       computed exactly on host, fp32)
    P = exp(PSUM) * G ;  S[n] = sum_k P ;  A = P / S
    (shifting logits by maxs*x_sq[n] bounds exp args; the gap to the true rowmax
     is < ~4 so S never underflows)
    psum_E[k, 0:D] += sum_n A[n,k]*xT[n,d]            (PE accumulates whole batch,
    psum_E[k, D]   += sum_n A[n,k]                     ones-column fused colsum)

x is uploaded twice -- fp8e4m3 (D,N) for the distance matmuls (the codewords are
tiny, |c| <= 1/sqrt(K*D), so fp8 error in <x,c> perturbs logits by < 1e-2) and
pre-transposed bf16 tiles (p, gi, d) for the aggregation matmuls.  Total HBM
traffic ~= 0.75 fp32 reads of x.  The mm2s of superblock s are emitted after the
softmax chain of superblock s+1 (software pipelining) and ping-pong between two
PSUM accumulators.  The (w-maxs) row is broadcast across partitions on-device via
a rank-1 matmul instead of uploading a pre-replicated 512KB tile.  P = exp(PSUM)*G
runs on DVE (flat bf16, fast); A = P/S runs on GpSimd to balance engine load.
"""
import os
import sys
import numpy as np

sys.path.insert(0, "/opt/trn_rl_repo")

import ml_dtypes  # noqa: E402

BF16 = ml_dtypes.bfloat16
FP8 = ml_dtypes.float8_e4m3

B, D, H, W = 32, 128, 128, 128
K = 32
N = H * W            # 16384
NCORES = 8
BPC = B // NCORES    # batches per core
TILN = 128           # n per tile (matmul stationary width)
NTIL = 16            # tiles per block
BLKN = TILN * NTIL   # 2048 n per block
NBLK = N // BLKN     # 8 blocks per batch

_CACHE = {}


def _build_module():
    from contextlib import ExitStack
    import concourse.tile as tile
    from concourse import bacc, mybir

    nc = bacc.Bacc("TRN2", target_bir_lowering=False, debug=False, num_devices=NCORES)
    bf = mybir.dt.bfloat16
    f8 = mybir.dt.float8e4
    f32 = mybir.dt.float32

    x_d = nc.dram_tensor("x", [BPC, D, N], f8, kind="ExternalInput").ap()
    # xt[b, p, gi, d] = x[b, d, gi*128 + p]
    xt_d = nc.dram_tensor("xt", [BPC, 128, N // TILN, D + 1], bf, kind="ExternalInput").ap()
    # xsqc[b, p, sib, j] = x_sq[b, n],  n = sib*4096 + j*128 + p
    xsqc_d = nc.dram_tensor("xsqc", [BPC, 128, N // 4096, 32], f32, kind="ExternalInput").ap()
    # wmk[0, k] = w[k] - maxs + w[k]*csq[k]/mean(xsq)  (f32; the csq/mean term
    # folds the tiny rank-1 w*csq logit seed into the G factor, error < 1e-4)
    wmk_d = nc.dram_tensor("wmk", [1, K], f32, kind="ExternalInput").ap()
    w1_d = nc.dram_tensor("w1", [D, K], bf, kind="ExternalInput").ap()
    oute_d = nc.dram_tensor("out_e", [BPC, K, 2, D + 1], f32, kind="ExternalOutput").ap()

    with tile.TileContext(nc) as tc, ExitStack() as ctx:
        cpool = ctx.enter_context(tc.tile_pool(name="const", bufs=1))
        xpool = ctx.enter_context(tc.tile_pool(name="xblk", bufs=4))
        xtpool = ctx.enter_context(tc.tile_pool(name="xtblk", bufs=4))
        qpool = ctx.enter_context(tc.tile_pool(name="xsqb", bufs=2))
        ppool = ctx.enter_context(tc.tile_pool(name="pexp", bufs=3))
        npool = ctx.enter_context(tc.tile_pool(name="pnorm", bufs=4))
        vpool = ctx.enter_context(tc.tile_pool(name="small", bufs=4))
        ps_xc = ctx.enter_context(tc.tile_pool(name="ps_xc", bufs=2, space="PSUM"))
        ps_e = ctx.enter_context(tc.tile_pool(name="ps_e", bufs=2, space="PSUM"))

        NSUP = 2                 # blocks per superblock load
        SUPN = BLKN * NSUP       # 4096 n per load chunk
        NSB = NBLK // NSUP       # superblocks per batch
        TPS = NTIL * NSUP        # 32 tiles per superblock

        # Tiny consts first (they gate the first matmul / t1), then the
        # superblock-0 bulk DMAs so compute can start ASAP.
        w1_sb = cpool.tile([D, K], bf)
        nc.sync.dma_start(out=w1_sb[:], in_=w1_d[:, :])
        wmk_in = cpool.tile([1, K], f32)
        nc.sync.dma_start(out=wmk_in[:], in_=wmk_d[:, :])
        # First superblock's x arrives as two half-tiles so the first mm1s
        # gate on 0.26MB instead of 0.52MB.
        x_sb0a = xpool.tile([D, SUPN // 2], f8)
        nc.sync.dma_start(out=x_sb0a[:], in_=x_d[0][:, 0 : SUPN // 2])
        x_sb0b = xpool.tile([D, SUPN // 2], f8)
        nc.sync.dma_start(out=x_sb0b[:], in_=x_d[0][:, SUPN // 2 : SUPN])
        xt_sb0 = xtpool.tile([128, TPS, D + 1], bf)
        nc.scalar.dma_start(out=xt_sb0[:], in_=xt_d[0][:, 0:TPS, :])
        # Broadcast wm across all 128 partitions (gpsimd, one-shot at startup).
        wmk_sb = cpool.tile([128, K], f32)
        nc.gpsimd.partition_broadcast(wmk_sb[:], wmk_in[:])

        # Software pipeline: mm2s of superblock s are emitted after the
        # softmax chain of superblock s+1, so the PE hides the chain latency.
        pending = []  # (b, sup_in_batch, pn_sb, xt_sb)
        psum_es = {}
        g_bs = {}
        first_mm2 = {}

        def emit_mm2s(b, sib, pn_halves, xt_sb):
            pe0, pe1 = psum_es[b]
            ff = first_mm2[b]
            for i in range(TPS):
                pp = i % 2
                pn_h = pn_halves[i // (TPS // 2)]
                nc.tensor.matmul(
                    (pe0, pe1)[pp][:],
                    lhsT=pn_h[:, K * (i % (TPS // 2)) : K * (i % (TPS // 2) + 1)],
                    rhs=xt_sb[:, i, :],
                    start=ff[pp],
                    stop=(sib == NSB - 1 and i >= TPS - 2),
                )
                ff[pp] = False
            if sib == NSB - 1:
                e_sb = vpool.tile([K, 2, D + 1], f32, tag="e_out")
                nc.scalar.copy(e_sb[:, 0, :], pe0[:])
                nc.scalar.copy(e_sb[:, 1, :], pe1[:])
                nc.sync.dma_start(out=oute_d[b], in_=e_sb[:])

        xsq_bs = {}

        def prologue_open(b):
            """Allocate batch-b G buffers and start its xsq DMA."""
            xsq_b = qpool.tile(
                [128, (N // 4096) * 32], f32, name=f"xsq_b{b}", tag="xsqb"
            )
            nc.sync.dma_start(
                out=xsq_b[:], in_=xsqc_d[b].rearrange("p s j -> p (s j)")
            )
            g_b = qpool.tile([128, N // 128 * K], bf, name=f"g_b{b}", tag="gb")
            xsq_bs[b] = xsq_b
            g_bs[b] = g_b

        t1_b0 = qpool.tile([128, TPS * K], f32, name="t1_b0", tag="t1b")

        def prologue_slice(b, s):
            """One superblock's worth of G = exp(wm_k*x_sq[n]).  Batch 0
            (just-in-time): DVE outer-product mult + ACT exp per superblock.
            Batches 1+: 8 per-k ACT Exp calls with per-partition scale
            wm_k, interleaved across the previous batch's superblocks --
            no DVE involvement, so the softmax chain never queues behind
            G-building on the vector engine."""
            xsq_b = xsq_bs[b]
            g3 = g_bs[b][:].rearrange("p (j k) -> p j k", k=K)
            if b == 0:
                sl = slice(s * TPS * K, (s + 1) * TPS * K)
                nc.vector.tensor_tensor(
                    t1_b0[:].rearrange("p (j k) -> p j k", k=K),
                    wmk_sb[:][:, None, :].broadcast_to([128, TPS, K]),
                    xsq_b[:, s * TPS : (s + 1) * TPS].broadcast_to([128, TPS, K]),
                    op=mybir.AluOpType.mult,
                )
                nc.scalar.activation(
                    g_bs[b][:, sl], t1_b0[:], mybir.ActivationFunctionType.Exp
                )
            else:
                for kk in range(8 * s, 8 * s + 8):
                    nc.scalar.activation(
                        g3[:, :, kk],
                        xsq_b[:, :],
                        mybir.ActivationFunctionType.Exp,
                        scale=wmk_sb[:, kk : kk + 1],
                    )

        for gsup in range(BPC * NSB):
            b, sib = divmod(gsup, NSB)
            if sib == 0:
                if b == 0:
                    prologue_open(0)
                psum_es[b] = (
                    ps_e.tile([K, D + 1], f32, tag="pe0", name=f"psum_e0_b{b}"),
                    ps_e.tile([K, D + 1], f32, tag="pe1", name=f"psum_e1_b{b}"),
                )
                first_mm2[b] = [True, True]
            if b == 0:
                # batch 0: G slice just-in-time for this superblock's chain
                prologue_slice(0, sib)
            if b + 1 < BPC and sib == 0:
                prologue_open(b + 1)
            soff = sib * SUPN
            if gsup == 0:
                x_parts, xt_sb = (x_sb0a, x_sb0b), xt_sb0
            else:
                x_sb = xpool.tile([D, SUPN], f8)
                nc.sync.dma_start(out=x_sb[:], in_=x_d[b][:, soff : soff + SUPN])
                x_parts = (x_sb,)
                xt_sb = xtpool.tile([128, TPS, D + 1], bf)
                nc.scalar.dma_start(
                    out=xt_sb[:], in_=xt_d[b][:, sib * TPS : (sib + 1) * TPS, :]
                )
            psum_xc = ps_xc.tile([128, TPS * K], f32)
            tpp = TPS // len(x_parts)
            for i in range(TPS):
                nc.tensor.matmul(
                    psum_xc[:, K * i : K * (i + 1)],
                    lhsT=x_parts[i // tpp][:, TILN * (i % tpp) : TILN * (i % tpp + 1)],
                    rhs=w1_sb[:, :],
                    start=True,
                    stop=True,
                    skip_group_check=True,
                )

            # Softmax chain in two half-superblock pieces so the halves
            # pipeline through ACT/DVE/GpSimd and the first half's A is
            # ready for mm2s ~2us sooner.  Normalize h0 on GpSimd, h1 on
            # DVE (parallel engines, both off the other half's path).
            HTK = TPS * K // 2   # columns per half
            HT = TPS // 2        # i-tiles per half
            pe_sb = ppool.tile([128, TPS * K], bf, tag="pexp")
            p_sb = ppool.tile([128, TPS * K], bf, tag="p")
            pn_halves = (
                npool.tile([128, HTK], bf, tag="pnA", name=f"pnA_{gsup}"),
                npool.tile([128, HTK], bf, tag="pnB", name=f"pnB_{gsup}"),
            )
            s_sb = vpool.tile([128, TPS], f32, tag="s")
            sinv_sb = vpool.tile([128, TPS], f32, tag="sinv")
            for h in range(2):
                hsl = slice(h * HTK, (h + 1) * HTK)
                nc.scalar.activation(
                    pe_sb[:, hsl], psum_xc[:, hsl],
                    mybir.ActivationFunctionType.Exp,
                )
                nc.vector.tensor_tensor(
                    p_sb[:, hsl],
                    pe_sb[:, hsl],
                    g_bs[b][:, sib * TPS * K + h * HTK : sib * TPS * K + (h + 1) * HTK],
                    op=mybir.AluOpType.mult,
                )
                p3h = p_sb[:, hsl].rearrange("p (i k) -> p i k", k=K)
                ssl = slice(h * HT, (h + 1) * HT)
                nc.vector.reduce_sum(s_sb[:, ssl], p3h, axis=mybir.AxisListType.X)
                nc.vector.reciprocal(sinv_sb[:, ssl], s_sb[:, ssl])
                norm_eng = nc.gpsimd if h == 0 else nc.vector
                norm_eng.tensor_tensor(
                    pn_halves[h][:].rearrange("p (i k) -> p i k", k=K),
                    p3h,
                    sinv_sb[:, ssl].broadcast_to([128, HT, K]),
                    op=mybir.AluOpType.mult,
                )

            if b + 1 < BPC:
                # batch b+1's G: one slice per superblock of batch b, queued
                # behind this superblock's chain so it fills engine idle time
                prologue_slice(b + 1, sib)

            pending.append((b, sib, pn_halves, xt_sb))
            if len(pending) > 2:
                emit_mm2s(*pending.pop(0))

        while pending:
            emit_mm2s(*pending.pop(0))

    nc.compile()
    return nc


def _get_module():
    if "nc" not in _CACHE:
        _CACHE["nc"] = _build_module()
    return _CACHE["nc"]


def _host_prep(x, codewords, scale):
    x = np.asarray(x, dtype=np.float32)
    c = np.asarray(codewords, dtype=np.float32)
    s = np.asarray(scale, dtype=np.float32)

    w = -s                           # (K,) in (0, 1)
    maxs = float(w.max())
    w1 = (-2.0 * (w[:, None] * c)).T.astype(BF16)           # (D, K)
    wcsq = w * (c * c).sum(axis=1)                          # (K,)

    xf = x.reshape(B, D, N)
    xsq = np.einsum("bdn,bdn->bn", xf, xf)                  # (B, N) fp32
    # xsqc[b, p, s, j] = xsq[b, s*4096 + j*128 + p]
    xsqc = np.ascontiguousarray(
        xsq.reshape(B, N // 4096, 32, 128).transpose(0, 3, 1, 2)
    )                                                       # (B, 128, N/4096, 32) f32
    # fold the rank-1 w*csq logit term into the G factor via its xsq-mean:
    # exp(wm*xsq + wcsq) ~= exp((wm + wcsq/mean(xsq))*xsq), |wcsq| <= 1e-2
    wm = (w - maxs) + wcsq / float(xsq.mean())              # (K,) ~<= 0
    wmk = wm[None, :].astype(np.float32)                    # (1, K)

    xb = xf.astype(FP8)                                     # (B, D, N) fp8e4m3
    # xt[b, p, gi, d] = xf[b, d, gi*128 + p];  xt[..., D] = 1.0 (fused colsum column)
    xt = np.ones((B, N // TILN, TILN, D + 1), dtype=BF16)
    xt[:, :, :, :D] = xf.transpose(0, 2, 1).reshape(B, N // TILN, TILN, D).astype(BF16)
    xt = np.ascontiguousarray(xt.transpose(0, 2, 1, 3))     # (B, 128, N/128, D+1)
    return xb, xt, xsqc, wmk, w1


def make_in_maps(x, codewords, scale):
    xb, xt, xsqc, wmk, w1 = _host_prep(x, codewords, scale)
    in_maps = []
    for ci in range(NCORES):
        sl = slice(BPC * ci, BPC * (ci + 1))
        in_maps.append(
            {
                "x": np.ascontiguousarray(xb[sl]),
                "xt": np.ascontiguousarray(xt[sl]),
                "xsqc": np.ascontiguousarray(xsqc[sl]),
                "wmk": wmk,
                "w1": w1,
            }
        )
    return in_maps


def finish_output(results, codewords):
    c = np.asarray(codewords, dtype=np.float32)
    out = np.zeros((B, K * D), dtype=np.float32)
    for ci, r in enumerate(results):
        for bb in range(BPC):
            e_parts = r["out_e"][bb][:, 0, :] + r["out_e"][bb][:, 1, :]   # (K, D+1)
            e = e_parts[:, :D] - e_parts[:, D : D + 1] * c
            out[BPC * ci + bb] = e.reshape(-1)
    return out


def kernel(x, codewords, scale):
    from concourse.bass_utils import run_bass_kernel_spmd
    from concourse.bass_interp import get_hw_module

    nc = _get_module()
    in_maps = make_in_maps(x, codewords, scale)

    old_m = nc.m
    nc.m = get_hw_module(nc.m)
    try:
        res = run_bass_kernel_spmd(nc, in_maps, core_ids=list(range(NCORES)))
    finally:
        nc.m = old_m
    return finish_output(res.results, codewords)


# revision 30
# speedup vs baseline: 1.0747x; 1.0747x over previous
"""DeepTEN encoding kernel for Trainium2 (8 NeuronCores, SPMD data-parallel over batch).

Math (per batch b):
    xf = x[b] viewed (D, N), N = H*W
    dist[n,k] = ||xf[:,n] - c[k]||^2 ;  logits = -scale * dist ;  A = softmax_k(logits)
    E[k,d] = sum_n A[n,k] * (xf[d,n] - c[k,d]) = (A^T X)[k,d] - colsum(A)[k]*c[k,d]

Device decomposition (softmax in (n-partitions, k-free) layout):
    w = -scale (>0), maxs = max(w)
    PSUM(xc)[n,k] = -2*w_k*<x_n,c_k> + w_k*csq_k      (x-tile-stationary matmuls +
                                                       a rank-1 seed matmul)
    G[n,k] = exp((w_k-maxs)*x_sq[n])                  (DVE mult + ACT exp; x_sq is
                                                       computed exactly on host, fp32)
    P = exp(PSUM) * G ;  S[n] = sum_k P ;  A = P / S
    (shifting logits by maxs*x_sq[n] bounds exp args; the gap to the true rowmax
     is < ~4 so S never underflows)
    psum_E[k, 0:D] += sum_n A[n,k]*xT[n,d]            (PE accumulates whole batch,
    psum_E[k, D]   += sum_n A[n,k]                     ones-column fused colsum)

x is uploaded twice -- fp8e4m3 (D,N) for the distance matmuls (the codewords are
tiny, |c| <= 1/sqrt(K*D), so fp8 error in <x,c> perturbs logits by < 1e-2) and
pre-transposed bf16 tiles (p, gi, d) for the aggregation matmuls.  Total HBM
traffic ~= 0.75 fp32 reads of x.  The mm2s of superblock s are emitted after the
softmax chain of superblock s+1 (software pipelining) and ping-pong between two
PSUM accumulators.  The (w-maxs) row is broadcast across partitions on-device via
a rank-1 matmul instead of uploading a pre-replicated 512KB tile.  P = exp(PSUM)*G
runs on DVE (flat bf16, fast); A = P/S runs on GpSimd to balance engine load.
"""
import os
import sys
import numpy as np

sys.path.insert(0, "/opt/trn_rl_repo")

import ml_dtypes  # noqa: E402

BF16 = ml_dtypes.bfloat16
FP8 = ml_dtypes.float8_e4m3

B, D, H, W = 32, 128, 128, 128
K = 32
N = H * W            # 16384
NCORES = 8
BPC = B // NCORES    # batches per core
TILN = 128           # n per tile (matmul stationary width)
NTIL = 16            # tiles per block
BLKN = TILN * NTIL   # 2048 n per block
NBLK = N // BLKN     # 8 blocks per batch

_CACHE = {}


def _build_module():
    from contextlib import ExitStack
    import concourse.tile as tile
    from concourse import bacc, mybir

    nc = bacc.Bacc("TRN2", target_bir_lowering=False, debug=False, num_devices=NCORES)
    bf = mybir.dt.bfloat16
    f8 = mybir.dt.float8e4
    f32 = mybir.dt.float32

    x_d = nc.dram_tensor("x", [BPC, D, N], f8, kind="ExternalInput").ap()
    # xt[b, p, gi, d] = x[b, d, gi*128 + p]
    xt_d = nc.dram_tensor("xt", [BPC, 128, N // TILN, D + 1], bf, kind="ExternalInput").ap()
    # xsqc[b, p, sib, j] = x_sq[b, n],  n = sib*4096 + j*128 + p
    xsqc_d = nc.dram_tensor("xsqc", [BPC, 128, N // 4096, 32], f32, kind="ExternalInput").ap()
    # wmk[0, k] = w[k] - maxs + w[k]*csq[k]/mean(xsq)  (f32; the csq/mean term
    # folds the tiny rank-1 w*csq logit seed into the G factor, error < 1e-4)
    wmk_d = nc.dram_tensor("wmk", [1, K], f32, kind="ExternalInput").ap()
    w1_d = nc.dram_tensor("w1", [D, K], bf, kind="ExternalInput").ap()
    oute_d = nc.dram_tensor("out_e", [BPC, K, 2, D + 1], f32, kind="ExternalOutput").ap()

    with tile.TileContext(nc) as tc, ExitStack() as ctx:
        cpool = ctx.enter_context(tc.tile_pool(name="const", bufs=1))
        xpool = ctx.enter_context(tc.tile_pool(name="xblk", bufs=4))
        xtpool = ctx.enter_context(tc.tile_pool(name="xtblk", bufs=4))
        qpool = ctx.enter_context(tc.tile_pool(name="xsqb", bufs=2))
        ppool = ctx.enter_context(tc.tile_pool(name="pexp", bufs=3))
        npool = ctx.enter_context(tc.tile_pool(name="pnorm", bufs=4))
        vpool = ctx.enter_context(tc.tile_pool(name="small", bufs=4))
        ps_xc = ctx.enter_context(tc.tile_pool(name="ps_xc", bufs=2, space="PSUM"))
        ps_e = ctx.enter_context(tc.tile_pool(name="ps_e", bufs=2, space="PSUM"))

        NSUP = 2                 # blocks per superblock load
        SUPN = BLKN * NSUP       # 4096 n per load chunk
        NSB = NBLK // NSUP       # superblocks per batch
        TPS = NTIL * NSUP        # 32 tiles per superblock

        # Tiny consts first (they gate the first matmul / t1), then the
        # superblock-0 bulk DMAs so compute can start ASAP.
        w1_sb = cpool.tile([D, K], bf)
        nc.sync.dma_start(out=w1_sb[:], in_=w1_d[:, :])
        wmk_in = cpool.tile([1, K], f32)
        nc.sync.dma_start(out=wmk_in[:], in_=wmk_d[:, :])
        # First superblock's x arrives as two half-tiles so the first mm1s
        # gate on 0.26MB instead of 0.52MB.
        x_sb0a = xpool.tile([D, SUPN // 2], f8)
        nc.sync.dma_start(out=x_sb0a[:], in_=x_d[0][:, 0 : SUPN // 2])
        x_sb0b = xpool.tile([D, SUPN // 2], f8)
        nc.sync.dma_start(out=x_sb0b[:], in_=x_d[0][:, SUPN // 2 : SUPN])
        xt_sb0 = xtpool.tile([128, TPS, D + 1], bf)
        nc.scalar.dma_start(out=xt_sb0[:], in_=xt_d[0][:, 0:TPS, :])
        # Broadcast wm across all 128 partitions (gpsimd, one-shot at startup).
        wmk_sb = cpool.tile([128, K], f32)
        nc.gpsimd.partition_broadcast(wmk_sb[:], wmk_in[:])

        # Software pipeline: mm2s of superblock s are emitted after the
        # softmax chain of superblock s+1, so the PE hides the chain latency.
        pending = []  # (b, sup_in_batch, pn_sb, xt_sb)
        psum_es = {}
        g_bs = {}
        first_mm2 = {}

        def emit_mm2s(b, sib, pn_halves, xt_sb):
            pe0, pe1 = psum_es[b]
            ff = first_mm2[b]
            for i in range(TPS):
                pp = i % 2
                pn_h = pn_halves[i // (TPS // 2)]
                nc.tensor.matmul(
                    (pe0, pe1)[pp][:],
                    lhsT=pn_h[:, K * (i % (TPS // 2)) : K * (i % (TPS // 2) + 1)],
                    rhs=xt_sb[:, i, :],
                    start=ff[pp],
                    stop=(sib == NSB - 1 and i >= TPS - 2),
                )
                ff[pp] = False
            if sib == NSB - 1:
                e_sb = vpool.tile([K, 2, D + 1], f32, tag="e_out")
                nc.scalar.copy(e_sb[:, 0, :], pe0[:])
                nc.scalar.copy(e_sb[:, 1, :], pe1[:])
                nc.sync.dma_start(out=oute_d[b], in_=e_sb[:])

        prologue_tiles = {}

        def prologue_open(b):
            """Allocate batch-b G buffers and start its xsq DMA."""
            xsq_b = qpool.tile(
                [128, (N // 4096) * 32], f32, name=f"xsq_b{b}", tag="xsqb"
            )
            nc.sync.dma_start(
                out=xsq_b[:], in_=xsqc_d[b].rearrange("p s j -> p (s j)")
            )
            t1_b = qpool.tile([128, N // 128 * K], f32, name=f"t1_b{b}", tag="t1b")
            g_b = qpool.tile([128, N // 128 * K], bf, name=f"g_b{b}", tag="gb")
            prologue_tiles[b] = (xsq_b, t1_b)
            g_bs[b] = g_b

        def prologue_slice(b, s):
            """One superblock's worth of G = exp(wm*x_sq): a ~1.1us DVE mult
            + ~0.9us ACT exp.  Slices for batch b+1 are interleaved across
            batch b's superblocks so the per-batch G build never lumps up in
            the DVE/ACT FIFOs ahead of the softmax chain."""
            xsq_b, t1_b = prologue_tiles[b]
            sl = slice(s * TPS * K, (s + 1) * TPS * K)
            nc.vector.tensor_tensor(
                t1_b[:, sl].rearrange("p (j k) -> p j k", k=K),
                wmk_sb[:][:, None, :].broadcast_to([128, TPS, K]),
                xsq_b[:, s * TPS : (s + 1) * TPS].broadcast_to([128, TPS, K]),
                op=mybir.AluOpType.mult,
            )
            nc.scalar.activation(
                g_bs[b][:, sl], t1_b[:, sl], mybir.ActivationFunctionType.Exp
            )

        for gsup in range(BPC * NSB):
            b, sib = divmod(gsup, NSB)
            if sib == 0:
                if b == 0:
                    prologue_open(0)
                psum_es[b] = (
                    ps_e.tile([K, D + 1], f32, tag="pe0", name=f"psum_e0_b{b}"),
                    ps_e.tile([K, D + 1], f32, tag="pe1", name=f"psum_e1_b{b}"),
                )
                first_mm2[b] = [True, True]
            if b == 0:
                # batch 0: G slice just-in-time for this superblock's chain
                prologue_slice(0, sib)
            if b + 1 < BPC and sib == 0:
                prologue_open(b + 1)
            soff = sib * SUPN
            if gsup == 0:
                x_parts, xt_sb = (x_sb0a, x_sb0b), xt_sb0
            else:
                x_sb = xpool.tile([D, SUPN], f8)
                nc.sync.dma_start(out=x_sb[:], in_=x_d[b][:, soff : soff + SUPN])
                x_parts = (x_sb,)
                xt_sb = xtpool.tile([128, TPS, D + 1], bf)
                nc.scalar.dma_start(
                    out=xt_sb[:], in_=xt_d[b][:, sib * TPS : (sib + 1) * TPS, :]
                )
            psum_xc = ps_xc.tile([128, TPS * K], f32)
            tpp = TPS // len(x_parts)
            for i in range(TPS):
                nc.tensor.matmul(
                    psum_xc[:, K * i : K * (i + 1)],
                    lhsT=x_parts[i // tpp][:, TILN * (i % tpp) : TILN * (i % tpp + 1)],
                    rhs=w1_sb[:, :],
                    start=True,
                    stop=True,
                    skip_group_check=True,
                )

            # Softmax chain in two half-superblock pieces so the halves
            # pipeline through ACT/DVE/GpSimd and the first half's A is
            # ready for mm2s ~2us sooner.  Normalize h0 on GpSimd, h1 on
            # DVE (parallel engines, both off the other half's path).
            HTK = TPS * K // 2   # columns per half
            HT = TPS // 2        # i-tiles per half
            pe_sb = ppool.tile([128, TPS * K], bf, tag="pexp")
            p_sb = ppool.tile([128, TPS * K], bf, tag="p")
            pn_halves = (
                npool.tile([128, HTK], bf, tag="pnA", name=f"pnA_{gsup}"),
                npool.tile([128, HTK], bf, tag="pnB", name=f"pnB_{gsup}"),
            )
            s_sb = vpool.tile([128, TPS], f32, tag="s")
            sinv_sb = vpool.tile([128, TPS], f32, tag="sinv")
            for h in range(2):
                hsl = slice(h * HTK, (h + 1) * HTK)
                half_eng = nc.gpsimd if h == 0 else nc.vector
                nc.scalar.activation(
                    pe_sb[:, hsl], psum_xc[:, hsl],
                    mybir.ActivationFunctionType.Exp,
                )
                half_eng.tensor_tensor(
                    p_sb[:, hsl],
                    pe_sb[:, hsl],
                    g_bs[b][:, sib * TPS * K + h * HTK : sib * TPS * K + (h + 1) * HTK],
                    op=mybir.AluOpType.mult,
                )
                p3h = p_sb[:, hsl].rearrange("p (i k) -> p i k", k=K)
                ssl = slice(h * HT, (h + 1) * HT)
                nc.vector.reduce_sum(s_sb[:, ssl], p3h, axis=mybir.AxisListType.X)
                nc.vector.reciprocal(sinv_sb[:, ssl], s_sb[:, ssl])
                half_eng.tensor_tensor(
                    pn_halves[h][:].rearrange("p (i k) -> p i k", k=K),
                    p3h,
                    sinv_sb[:, ssl].broadcast_to([128, HT, K]),
                    op=mybir.AluOpType.mult,
                )

            if b + 1 < BPC:
                # batch b+1's G: one slice per superblock of batch b, queued
                # behind this superblock's chain so it fills engine idle time
                prologue_slice(b + 1, sib)

            pending.append((b, sib, pn_halves, xt_sb))
            if len(pending) > 2:
                emit_mm2s(*pending.pop(0))

        while pending:
            emit_mm2s(*pending.pop(0))

    nc.compile()
    return nc


def _get_module():
    if "nc" not in _CACHE:
        _CACHE["nc"] = _build_module()
    return _CACHE["nc"]


def _host_prep(x, codewords, scale):
    x = np.asarray(x, dtype=np.float32)
    c = np.asarray(codewords, dtype=np.float32)
    s = np.asarray(scale, dtype=np.float32)

    w = -s                           # (K,) in (0, 1)
    maxs = float(w.max())
    w1 = (-2.0 * (w[:, None] * c)).T.astype(BF16)           # (D, K)
    wcsq = w * (c * c).sum(axis=1)                          # (K,)

    xf = x.reshape(B, D, N)
    xsq = np.einsum("bdn,bdn->bn", xf, xf)                  # (B, N) fp32
    # xsqc[b, p, s, j] = xsq[b, s*4096 + j*128 + p]
    xsqc = np.ascontiguousarray(
        xsq.reshape(B, N // 4096, 32, 128).transpose(0, 3, 1, 2)
    )                                                       # (B, 128, N/4096, 32) f32
    # fold the rank-1 w*csq logit term into the G factor via its xsq-mean:
    # exp(wm*xsq + wcsq) ~= exp((wm + wcsq/mean(xsq))*xsq), |wcsq| <= 1e-2
    wm = (w - maxs) + wcsq / float(xsq.mean())              # (K,) ~<= 0
    wmk = wm[None, :].astype(np.float32)                    # (1, K)

    xb = xf.astype(FP8)                                     # (B, D, N) fp8e4m3
    # xt[b, p, gi, d] = xf[b, d, gi*128 + p];  xt[..., D] = 1.0 (fused colsum column)
    xt = np.ones((B, N // TILN, TILN, D + 1), dtype=BF16)
    xt[:, :, :, :D] = xf.transpose(0, 2, 1).reshape(B, N // TILN, TILN, D).astype(BF16)
    xt = np.ascontiguousarray(xt.transpose(0, 2, 1, 3))     # (B, 128, N/128, D+1)
    return xb, xt, xsqc, wmk, w1


def make_in_maps(x, codewords, scale):
    xb, xt, xsqc, wmk, w1 = _host_prep(x, codewords, scale)
    in_maps = []
    for ci in range(NCORES):
        sl = slice(BPC * ci, BPC * (ci + 1))
        in_maps.append(
            {
                "x": np.ascontiguousarray(xb[sl]),
                "xt": np.ascontiguousarray(xt[sl]),
                "xsqc": np.ascontiguousarray(xsqc[sl]),
                "wmk": wmk,
                "w1": w1,
            }
        )
    return in_maps


def finish_output(results, codewords):
    c = np.asarray(codewords, dtype=np.float32)
    out = np.zeros((B, K * D), dtype=np.float32)
    for ci, r in enumerate(results):
        for bb in range(BPC):
            e_parts = r["out_e"][bb][:, 0, :] + r["out_e"][bb][:, 1, :]   # (K, D+1)
            e = e_parts[:, :D] - e_parts[:, D : D + 1] * c
            out[BPC * ci + bb] = e.reshape(-1)
    return out


def kernel(x, codewords, scale):
    from concourse.bass_utils import run_bass_kernel_spmd
    from concourse.bass_interp import get_hw_module

    nc = _get_module()
    in_maps = make_in_maps(x, codewords, scale)

    old_m = nc.m
    nc.m = get_hw_module(nc.m)
    try:
        res = run_bass_kernel_spmd(nc, in_maps, core_ids=list(range(NCORES)))
    finally:
        nc.m = old_m
    return finish_output(res.results, codewords)
